# revision 9
# baseline (speedup 1.0000x reference)
"""Trainium2 Bass kernel for nn_ContrastiveLoss (NT-Xent style contrastive loss).

Strategy (8 NeuronCores, SPMD):
  - Host sorts samples by label (the scalar loss is permutation invariant),
    row-normalizes, and builds X^T [D=128, N=8192] in bf16.
  - Rows are sharded across 8 cores (1024 rows each, 8 blocks of 128).
  - Each core computes its [1024, 8192] similarity block against the full
    X^T (the "all-gathered" copy arrives as a per-core input), reduces
    exp-row-sums on-chip, and evaluates the positive-pair terms only on a
    narrow label-band window (sorted labels make positives contiguous).
  - Per-row partial losses return to the host, which sums them and divides
    by the exact positive-pair count (from the label histogram).

Math: with e_ij = exp(sim_ij/T), S_i = sum_j e_ij (incl diag),
P_i = sum_{j in label-range(i)} e_ij (incl diag), unsim_i = S_i - P_i,
u_i = log(unsim_i), the reference loss row-sum equals
  npos_i*u_i + sum_{range} softplus(sim_ij/T - u_i) - softplus(1/T - u_i)
             - (sum_{range} sim_ij/T - 1/T)
where npos_i = (label count of i) - 1. The diagonal contributions cancel
exactly in unsim and are removed via the constant sim_ii = 1 (rows are
normalized; the fp difference is ~1e-9 relative on the final scalar).
"""

import numpy as np

T = 0.2
INV_T = 1.0 / T  # 5.0
EPS = 1e-5
N, D, NCLASS = 8192, 128, 128
NCORES = 8
ROWS_PER_CORE = N // NCORES          # 1024
BLOCKS = ROWS_PER_CORE // 128        # 8 blocks of 128 rows per core
CHUNK = 2048                         # ACT chunk (4 PSUM banks)
NCHUNKS = N // CHUNK                 # 4 per block
MM = 512                             # matmul free-dim per PSUM bank

_CACHE = {}


def _build_nc(W, debug=False):
    """Build the SPMD Bass/Tile program. W = band window width (mult of 512)."""
    import concourse.bass as bass
    import concourse.bacc as bacc
    import concourse.mybir as mybir
    import concourse.tile as tile

    dt = mybir.dt
    AF = mybir.ActivationFunctionType
    ALU = mybir.AluOpType
    X = mybir.AxisListType.X

    nc = bacc.Bacc("TRN2", target_bir_lowering=False, debug=debug)

    xt_d = nc.dram_tensor("xt", [128, N], dt.bfloat16, kind="ExternalInput")
    xtown_d = nc.dram_tensor("xtown", [128, ROWS_PER_CORE], dt.bfloat16,
                             kind="ExternalInput")
    xtband_d = nc.dram_tensor("xtband", [128, BLOCKS * W], dt.bfloat16,
                              kind="ExternalInput")
    gsr_d = nc.dram_tensor("gsr", [128, BLOCKS], dt.float32, kind="ExternalInput")
    ger_d = nc.dram_tensor("ger", [128, BLOCKS], dt.float32, kind="ExternalInput")
    npos_d = nc.dram_tensor("npos", [128, BLOCKS], dt.float32, kind="ExternalInput")
    out_d = nc.dram_tensor("out", [128, BLOCKS], dt.float32, kind="ExternalOutput")

    nwc = W // MM  # band matmul sub-chunks

    with tile.TileContext(nc) as tc:
        with (
            tc.tile_pool(name="const", bufs=1) as const,
            tc.tile_pool(name="band", bufs=1) as band,
            tc.tile_pool(name="etmp", bufs=3) as etmp_pool,
            tc.tile_pool(name="sp", bufs=2) as sp_pool,
            tc.tile_pool(name="small", bufs=1) as small,
            tc.tile_pool(name="psum", bufs=2, space="PSUM") as psum,
        ):
            # ---- persistent loads ----
            xt = const.tile([128, N], dt.bfloat16)
            for k in range(N // CHUNK):
                nc.sync.dma_start(xt[:, k * CHUNK:(k + 1) * CHUNK],
                                  xt_d[:, k * CHUNK:(k + 1) * CHUNK])
            xtown = const.tile([128, ROWS_PER_CORE], dt.bfloat16)
            nc.sync.dma_start(xtown[:], xtown_d[:])
            xtband = const.tile([128, BLOCKS * W], dt.bfloat16)
            nc.sync.dma_start(xtband[:], xtband_d[:])
            gsr = const.tile([128, BLOCKS], dt.float32)
            nc.sync.dma_start(gsr[:], gsr_d[:])
            ger = const.tile([128, BLOCKS], dt.float32)
            nc.sync.dma_start(ger[:], ger_d[:])
            npos = const.tile([128, BLOCKS], dt.float32)
            nc.sync.dma_start(npos[:], npos_d[:])

            iota_i = const.tile([128, W], dt.int32)
            nc.gpsimd.iota(iota_i[:], pattern=[[1, W]], base=0, channel_multiplier=0)
            iota_f = const.tile([128, W], dt.float32)
            nc.vector.tensor_copy(iota_f[:], iota_i[:])

            acc = const.tile([128, BLOCKS], dt.float32)
            five = const.tile([128, 1], dt.float32)
            nc.vector.memset(five[:], INV_T)

            # per-block persistent tiles
            s_band = [band.tile([128, W], dt.float32, name=f"sb{b}") for b in range(BLOCKS)]
            e_band = [band.tile([128, W], dt.float32, name=f"eb{b}") for b in range(BLOCKS)]
            mask = [band.tile([128, W], dt.float32, name=f"mk{b}") for b in range(BLOCKS)]
            S = [small.tile([128, 1], dt.float32, name=f"S{b}") for b in range(BLOCKS)]
            P = [small.tile([128, 1], dt.float32, name=f"P{b}") for b in range(BLOCKS)]
            u = [small.tile([128, 1], dt.float32, name=f"u{b}") for b in range(BLOCKS)]
            negu = [small.tile([128, 1], dt.float32, name=f"nu{b}") for b in range(BLOCKS)]
            spd = [small.tile([128, 1], dt.float32, name=f"sd{b}") for b in range(BLOCKS)]
            sparts = [small.tile([128, NCHUNKS], dt.float32, name=f"sp{b}")
                      for b in range(BLOCKS)]

            # ---- Phase A: dense exp row-sums (Exp table) + band sims ----
            for b in range(BLOCKS):
                lhsT = xtown[:, b * 128:(b + 1) * 128]
                for kc in range(NCHUNKS):
                    ps = psum.tile([128, CHUNK], dt.float32, tag="ps")
                    for j in range(CHUNK // MM):
                        c0 = kc * CHUNK + j * MM
                        nc.tensor.matmul(ps[:, j * MM:(j + 1) * MM], lhsT,
                                         xt[:, c0:c0 + MM], start=True, stop=True)
                    e_tmp = etmp_pool.tile([128, CHUNK], dt.float32, tag="et")
                    nc.scalar.activation(e_tmp[:], ps[:], AF.Exp, bias=0.0,
                                         scale=INV_T,
                                         accum_out=sparts[b][:, kc:kc + 1])
                # band: sims for the W-wide positive window
                psb = psum.tile([128, W], dt.float32, tag="ps")
                for j in range(nwc):
                    nc.tensor.matmul(psb[:, j * MM:(j + 1) * MM], lhsT,
                                     xtband[:, b * W + j * MM: b * W + (j + 1) * MM],
                                     start=True, stop=True)
                nc.scalar.activation(e_band[b][:], psb[:], AF.Exp, bias=0.0,
                                     scale=INV_T)
                nc.vector.tensor_copy(s_band[b][:], psb[:])
                nc.vector.reduce_sum(S[b][:], sparts[b][:], axis=X)

            # ---- Phase B: range masks + positive-window sums (DVE only) ----
            tmp_pool = sp_pool
            for b in range(BLOCKS):
                m1 = tmp_pool.tile([128, W], dt.float32, tag="m1")
                nc.vector.tensor_scalar(m1[:], iota_f[:], gsr[:, b:b + 1], None,
                                        op0=ALU.is_ge)
                nc.vector.scalar_tensor_tensor(mask[b][:], iota_f[:],
                                               ger[:, b:b + 1], m1[:],
                                               op0=ALU.is_lt, op1=ALU.mult)
                ttmp = tmp_pool.tile([128, W], dt.float32, tag="tt")
                nc.vector.tensor_mul(ttmp[:], e_band[b][:], mask[b][:])
                nc.vector.reduce_sum(P[b][:], ttmp[:], axis=X)
                # unsim = S - P  (reuse P tile as unsim)
                nc.vector.tensor_sub(P[b][:], S[b][:], P[b][:])

            # ---- Phase C: u = log(unsim) (Ln table) ----
            for b in range(BLOCKS):
                nc.scalar.activation(u[b][:], P[b][:], AF.Ln)
            for b in range(BLOCKS):
                nc.vector.tensor_scalar_mul(negu[b][:], u[b][:], -1.0)

            # ---- Phase D: softplus terms via Ln(1+Exp(z)) (same table set) ----
            sp_tiles = []
            for b in range(BLOCKS):
                t2 = small.tile([128, 1], dt.float32, name=f"t2{b}")
                nc.scalar.activation(t2[:], u[b][:], AF.Exp,
                                     bias=five[:], scale=-1.0)
                nc.scalar.activation(spd[b][:], t2[:], AF.Ln, bias=1.0)
                et = sp_pool.tile([128, W], dt.float32, tag="spe")
                nc.scalar.activation(et[:], s_band[b][:], AF.Exp,
                                     bias=negu[b][:], scale=INV_T)
                sp = sp_pool.tile([128, W], dt.float32, tag="spt")
                nc.scalar.activation(sp[:], et[:], AF.Ln, bias=1.0)
                sp_tiles.append(sp)

                # ---- Phase E interleaved (DVE): A, B, combine ----
                A = small.tile([128, 1], dt.float32, name=f"A{b}")
                B = small.tile([128, 1], dt.float32, name=f"B{b}")
                ttmp = tmp_pool.tile([128, W], dt.float32, tag="tt")
                nc.vector.tensor_mul(ttmp[:], sp[:], mask[b][:])
                nc.vector.reduce_sum(A[:], ttmp[:], axis=X)
                ttmp2 = tmp_pool.tile([128, W], dt.float32, tag="tt")
                nc.vector.tensor_mul(ttmp2[:], s_band[b][:], mask[b][:])
                nc.vector.reduce_sum(B[:], ttmp2[:], axis=X)
                # loss = npos*u + A - spd - (INV_T*B - INV_T)
                r1 = small.tile([128, 1], dt.float32, name=f"r1{b}")
                nc.vector.scalar_tensor_tensor(r1[:], u[b][:], npos[:, b:b + 1],
                                               A[:], op0=ALU.mult, op1=ALU.add)
                r2 = small.tile([128, 1], dt.float32, name=f"r2{b}")
                nc.vector.tensor_scalar(r2[:], B[:], INV_T, -INV_T,
                                        op0=ALU.mult, op1=ALU.add)
                r3 = small.tile([128, 1], dt.float32, name=f"r3{b}")
                nc.vector.tensor_add(r3[:], r2[:], spd[b][:])
                nc.vector.tensor_sub(acc[:, b:b + 1], r1[:], r3[:])

            nc.sync.dma_start(out_d[:], acc[:])

    nc.compile()
    return nc


def _prep(input, label):
    """Host-side shard prep: sort by label, normalize, build per-core inputs."""
    import ml_dtypes

    x = np.asarray(input, dtype=np.float32).reshape(N, D)
    lab = np.asarray(label).astype(np.int64).reshape(N)

    order = np.argsort(lab, kind="stable")
    xs, ls = x[order], lab[order]
    counts = np.bincount(ls, minlength=NCLASS)
    n_pos = int((counts.astype(np.int64) ** 2).sum()) - N
    ends = np.cumsum(counts)
    starts = ends - counts
    row_gs = starts[ls]          # [N] group start col per (sorted) row
    row_ge = ends[ls]            # [N] group end col per row

    norms = np.sqrt((xs * xs).sum(1, dtype=np.float32)).astype(np.float32)
    # reference divides by max(n_i*n_j, EPS); for this data the max never
    # binds (norms ~ 11), so plain normalization is exact.
    assert float(norms.min()) ** 2 > EPS * 1.0001
    xn = (xs / norms[:, None]).astype(np.float32)
    xt = np.ascontiguousarray(xn.T).astype(ml_dtypes.bfloat16)  # [128, N]

    # band windows per global block
    nblk = N // 128
    lo = row_gs[np.arange(nblk) * 128]
    hi = row_ge[np.arange(nblk) * 128 + 127]
    maxband = int((hi - lo).max())
    W = max(512, ((maxband + 511) // 512) * 512)
    wstart = np.minimum(lo, N - W)

    in_maps = []
    for c in range(NCORES):
        r0 = c * ROWS_PER_CORE
        xtband = np.empty((128, BLOCKS * W), dtype=ml_dtypes.bfloat16)
        gsr = np.empty((128, BLOCKS), np.float32)
        ger = np.empty((128, BLOCKS), np.float32)
        npos = np.empty((128, BLOCKS), np.float32)
        for b in range(BLOCKS):
            g = c * BLOCKS + b
            ws = int(wstart[g])
            xtband[:, b * W:(b + 1) * W] = xt[:, ws:ws + W]
            rows = slice(r0 + b * 128, r0 + (b + 1) * 128)
            gsr[:, b] = (row_gs[rows] - ws).astype(np.float32)
            ger[:, b] = (row_ge[rows] - ws).astype(np.float32)
            npos[:, b] = (row_ge[rows] - row_gs[rows] - 1).astype(np.float32)
        in_maps.append({
            "xt": xt,
            "xtown": np.ascontiguousarray(
                xt[:, r0:r0 + ROWS_PER_CORE]),
            "xtband": xtband,
            "gsr": gsr,
            "ger": ger,
            "npos": npos,
        })
    return in_maps, n_pos, W


def kernel(input, label):
    from concourse.bass_utils import run_bass_kernel_spmd

    in_maps, n_pos, W = _prep(input, label)
    if W not in _CACHE:
        _CACHE[W] = _build_nc(W)
    nc = _CACHE[W]

    res = None
    for attempt in range(4):
        try:
            res = run_bass_kernel_spmd(nc, in_maps, core_ids=list(range(NCORES)))
            break
        except Exception:
            if attempt == 3:
                raise
            import time
            time.sleep(45)  # device may need a moment to recover
    global LAST_RESULTS
    LAST_RESULTS = res
    total = 0.0
    for r in res.results:
        total += float(np.sum(r["out"], dtype=np.float64))
    return np.array(total / n_pos, dtype=np.float32)


LAST_RESULTS = None
